# revision 1
# baseline (speedup 1.0000x reference)
"""CP(n) lattice action kernel for Trainium2 (8 NeuronCores, Bass/Tile).

Math (matches reference):
  phi: (B=1024, S=4096, n=6) angles; shift: (2, S) neighbor site indices.
  Wrap: first 5 angles mod pi, last mod 2pi.
  x = hyperspherical embedding (7 comps); z = (x0..x3) + i(x4,x5,x6,0).
  w_d(s) = sum_k z_k(s) z_k(shift[d,s])
  action[b] = -4 * sum_{d,s} (|w_d(s)|^2 - 1)

Implementation notes:
  - Pure data parallel: batch axis sharded 8 x 128; batch on SBUF partitions,
    sites along the free dimension.
  - This platform is per-instruction-overhead dominated, so the kernel is
    built from few, large, multi-row instructions.
  - Angle wrap via identities (no floating mod on TRN2):
      phir = phi - 2pi*round(phi/2pi)  (round via fp->int RNE convert)
      sigma = Sign(phir)
      s_j  = sigma*Sin(phir) (j<5);  s_5 = Sin(phir)
      c_j  = Sin(sigma*pi/2 - phir) = sigma*cos(phir) (j<5); c_5 = sigma*that
  - Neighbor gather: shift values are read on host at build time; the
    roll-structured shift (nearest neighbor on a 64x64 lattice) lowers to
    offset/strided access patterns; arbitrary shift falls back to per-run
    copies.
  - Per-site |w|^2 and the site reduction fuse into ACT Square + accum_out.
"""
import contextlib
import sys

import numpy as np

sys.path.insert(0, "/opt/trn_rl_repo")

B, S, NA = 1024, 4096, 6
NCORES = 8
PB = B // NCORES          # 128 batches per core
C1 = 2048                 # stage-1 site chunk
N1 = S // C1
C2 = 2048                 # stage-2 site chunk
N2 = S // C2
L = 64                    # lattice row length
PI = float(np.pi)
NBETA = 4.0               # N * BETA

_cache = {}


def _detect_roll(shift):
    idx = np.arange(S).reshape(L, L)
    s0 = np.roll(idx, -1, axis=0).ravel()
    s1 = np.roll(idx, -1, axis=1).ravel()
    return np.array_equal(shift[0], s0) and np.array_equal(shift[1], s1)


def _runs(perm):
    runs = []
    st = 0
    for i in range(1, len(perm) + 1):
        if i == len(perm) or perm[i] != perm[i - 1] + 1:
            runs.append((st, int(perm[st]), i - st))
            st = i
    return runs


def _build(shift, reps=1, mode="full"):
    import concourse.bass as bass
    import concourse.tile as tile
    from concourse import bacc, mybir

    f32 = mybir.dt.float32
    bf16 = mybir.dt.bfloat16
    i16 = mybir.dt.int16
    Act = mybir.ActivationFunctionType
    Op = mybir.AluOpType
    X = mybir.AxisListType.X

    roll = _detect_roll(shift)

    nc = bacc.Bacc(None, target_bir_lowering=False)
    phi_d = nc.dram_tensor("phi", [PB, S, NA], f32, kind="ExternalInput")
    out_d = nc.dram_tensor("out", [PB, 1], f32, kind="ExternalOutput")
    pd_flat = phi_d[:].rearrange("p s a -> p (s a)")

    with tile.TileContext(nc) as tc:
        with contextlib.ExitStack() as ctx:
            xfull_pool = ctx.enter_context(tc.tile_pool(name="xfull", bufs=1))
            small_pool = ctx.enter_context(tc.tile_pool(name="small", bufs=1))

            NACC = N2
            acc = small_pool.tile([PB, NACC], f32)
            if roll:
                xf = xfull_pool.tile([PB, 7, S], bf16)
                xg = None
            else:
                # site-major cells [site, 8] so gpsimd.ap_gather can fetch
                # whole 7-component cells per shift index
                xf = None
                xg = xfull_pool.tile([PB, S, 8], bf16)
                idx_sb = []
                for d in range(2):
                    wrapped = np.zeros((PB, S // 16), np.int16)
                    base = shift[d].reshape(S // 16, 16).T.astype(np.int16)
                    for g in range(PB // 16):
                        wrapped[16 * g:16 * (g + 1)] = base
                    hdl = nc.inline_tensor(wrapped, name=f"shift_idx_{d}")
                    t_ = small_pool.tile([PB, S // 16], mybir.dt.int16, tag=f"idx{d}")
                    nc.sync.dma_start(t_[:], hdl[:])
                    idx_sb.append(t_)

            for rep in range(reps):
                # ======== stage 1: wrap + trig + embedding ========
                st1 = contextlib.ExitStack()
                p_phi = st1.enter_context(tc.tile_pool(name="p_phi", bufs=1))
                p_ks = st1.enter_context(tc.tile_pool(name="p_ks", bufs=1))
                p_t = st1.enter_context(tc.tile_pool(name="p_t", bufs=1))
                p_u = st1.enter_context(tc.tile_pool(name="p_u", bufs=1))
                p_cum = st1.enter_context(tc.tile_pool(name="p_cum", bufs=1))

                for ch in range(N1):
                    cs = ch * C1
                    M = C1 * NA

                    phic = p_phi.tile([PB, M], f32, tag="phic")
                    nc.sync.dma_start(phic[:], pd_flat[:, cs * NA:(cs + C1) * NA])

                    if mode == "dma":
                        nc.vector.tensor_reduce(acc[:, 0:1], phic[:, 0:8],
                                                axis=X, op=Op.add)
                        continue

                    # k = round(phi/2pi) as int16
                    k = p_ks.tile([PB, M], i16, tag="ks")
                    nc.vector.tensor_scalar(k[:], phic[:], 1.0 / (2 * PI), None,
                                            op0=Op.mult)
                    # phir = (k * -2pi) + phi   (in place)
                    nc.vector.scalar_tensor_tensor(
                        phic[:], k[:], -2 * PI, phic[:], op0=Op.mult, op1=Op.add)

                    # sigma, t = Sin(phir)  (interleaved site-major, bf16)
                    sig = p_ks.tile([PB, M], bf16, tag="ks")
                    nc.scalar.activation(sig[:], phic[:], Act.Sign)
                    tt = p_t.tile([PB, M], bf16, tag="t")
                    nc.scalar.activation(tt[:], phic[:], Act.Sin)
                    # arg2 = sigma*pi/2 - phir (in place over phir)
                    nc.vector.scalar_tensor_tensor(
                        phic[:], sig[:], PI / 2, phic[:],
                        op0=Op.mult, op1=Op.subtract)
                    # u = Sin(arg2) = sigma*cos(phir)
                    uu = p_u.tile([PB, M], bf16, tag="u")
                    nc.scalar.activation(uu[:], phic[:], Act.Sin)

                    def ang(tile_, j, n=1):
                        ap = tile_[:]
                        if n == 1:
                            return bass.AP(tensor=ap.tensor, offset=ap.offset + j,
                                           ap=[ap.ap[0], [NA, C1]])
                        return bass.AP(tensor=ap.tensor, offset=ap.offset + j,
                                       ap=[ap.ap[0], [NA, C1], [1, n]])

                    # s_j = sigma*t for j<5 (in place on t)
                    nc.vector.tensor_tensor(ang(tt, 0, 5), ang(tt, 0, 5),
                                            ang(sig, 0, 5), op=Op.mult)
                    # c_5 = sigma*u at j=5 (in place on u)
                    nc.vector.tensor_tensor(ang(uu, 5), ang(uu, 5),
                                            ang(sig, 5), op=Op.mult)

                    # cumprod + x build into xf rows / xg cells
                    cumA = p_cum.tile([PB, C1], bf16, tag="cumA")
                    cumB = p_cum.tile([PB, C1], bf16, tag="cumB")
                    if roll:
                        xs = xf[:, :, cs:cs + C1]
                        xk = [xs[:, k, :] for k in range(7)]
                    else:
                        gap = xg[:]
                        xk = [bass.AP(tensor=gap.tensor,
                                      offset=gap.offset + cs * 8 + k,
                                      ap=[gap.ap[0], [8, C1]])
                              for k in range(7)]
                    nc.vector.tensor_copy(xk[0], ang(uu, 0))
                    nc.vector.tensor_tensor(xk[1], ang(uu, 1), ang(tt, 0),
                                            op=Op.mult)
                    nc.vector.tensor_tensor(cumA[:], ang(tt, 0), ang(tt, 1),
                                            op=Op.mult)
                    nc.vector.tensor_tensor(xk[2], ang(uu, 2), cumA[:],
                                            op=Op.mult)
                    nc.vector.tensor_tensor(cumB[:], cumA[:], ang(tt, 2),
                                            op=Op.mult)
                    nc.vector.tensor_tensor(xk[3], ang(uu, 3), cumB[:],
                                            op=Op.mult)
                    nc.vector.tensor_tensor(cumA[:], cumB[:], ang(tt, 3),
                                            op=Op.mult)
                    nc.vector.tensor_tensor(xk[4], ang(uu, 4), cumA[:],
                                            op=Op.mult)
                    nc.vector.tensor_tensor(cumB[:], cumA[:], ang(tt, 4),
                                            op=Op.mult)
                    nc.vector.tensor_tensor(xk[5], ang(uu, 5), cumB[:],
                                            op=Op.mult)
                    nc.vector.tensor_tensor(xk[6], cumB[:], ang(tt, 5),
                                            op=Op.mult)

                st1.close()
                if mode in ("dma", "stage1"):
                    continue

                # ======== stage 2: neighbor products ========
                st2 = contextlib.ExitStack()
                p_xp = st2.enter_context(tc.tile_pool(name="p_xp", bufs=1))
                p_m = st2.enter_context(tc.tile_pool(name="p_m", bufs=1))
                p_pq = st2.enter_context(tc.tile_pool(name="p_pq", bufs=1))

                for ch in range(N2):
                    cs = ch * C2
                    if roll:
                        xs = xf[:, :, cs:cs + C2]
                    else:
                        gap = xg[:]
                        xs = None
                        xg_k = lambda k0, n, off=0: bass.AP(
                            tensor=gap.tensor,
                            offset=gap.offset + cs * 8 + k0,
                            ap=[gap.ap[0], [1, n], [8, C2]])

                    # double-width: both dirs side by side, shared folds
                    m = p_m.tile([PB, 7, 2 * C2], bf16, tag="m")
                    pq = p_pq.tile([PB, 6, 2 * C2], bf16, tag="pq")

                    for d in (0, 1):
                        if roll and d == 0:
                            lo = cs + L
                            if lo + C2 <= S:
                                xp_ap = xf[:, :, lo:lo + C2]
                            else:
                                xp = p_xp.tile([PB, 7, C2], bf16, tag="xp")
                                mn = S - lo
                                nc.vector.tensor_copy(xp[:, :, 0:mn],
                                                      xf[:, :, lo:S])
                                nc.vector.tensor_copy(xp[:, :, mn:C2],
                                                      xf[:, :, 0:C2 - mn])
                                xp_ap = xp[:]
                        elif roll and d == 1:
                            xp = p_xp.tile([PB, 7, C2], bf16, tag="xp")
                            nrow = C2 // L
                            src = bass.AP(
                                tensor=xf.tensor, offset=xf[:].offset + cs + 1,
                                ap=[xf[:].ap[0], [S, 7], [L, nrow], [1, L - 1]])
                            dst = bass.AP(
                                tensor=xp.tensor, offset=xp[:].offset,
                                ap=[xp[:].ap[0], [C2, 7], [L, nrow], [1, L - 1]])
                            nc.gpsimd.tensor_copy(dst, src)
                            srcw = bass.AP(
                                tensor=xf.tensor, offset=xf[:].offset + cs,
                                ap=[xf[:].ap[0], [S, 7], [L, nrow]])
                            dstw = bass.AP(
                                tensor=xp.tensor, offset=xp[:].offset + L - 1,
                                ap=[xp[:].ap[0], [C2, 7], [L, nrow]])
                            nc.gpsimd.tensor_copy(dstw, srcw)
                            xp_ap = xp[:]
                        else:
                            xpg = p_xp.tile([PB, C2, 8], bf16, tag="xp")
                            nc.gpsimd.ap_gather(
                                xpg[:], xg[:],
                                idx_sb[d][:, cs // 16:(cs + C2) // 16],
                                channels=PB, num_elems=S, d=8, num_idxs=C2)
                            gp = xpg[:]
                            xp_k = lambda k0, n: bass.AP(
                                tensor=gp.tensor, offset=gp.offset + k0,
                                ap=[gp.ap[0], [1, n], [8, C2]])

                        ms = m[:, :, d * C2:(d + 1) * C2]
                        pqs = pq[:, :, d * C2:(d + 1) * C2]
                        if roll:
                            nc.vector.tensor_tensor(ms, xs, xp_ap, op=Op.mult)
                            nc.vector.tensor_tensor(
                                pqs[:, 0:3, :], xs[:, 0:3, :],
                                xp_ap[:, 4:7, :], op=Op.mult)
                            nc.vector.tensor_tensor(
                                pqs[:, 3:6, :], xs[:, 4:7, :],
                                xp_ap[:, 0:3, :], op=Op.mult)
                        else:
                            nc.vector.tensor_tensor(ms, xg_k(0, 7),
                                                    xp_k(0, 7), op=Op.mult)
                            nc.vector.tensor_tensor(pqs[:, 0:3, :], xg_k(0, 3),
                                                    xp_k(4, 3), op=Op.mult)
                            nc.vector.tensor_tensor(pqs[:, 3:6, :], xg_k(4, 3),
                                                    xp_k(0, 3), op=Op.mult)

                    # shared folds across both dirs (double width)
                    nc.vector.tensor_tensor(m[:, 0:3, :], m[:, 0:3, :],
                                            m[:, 4:7, :], op=Op.subtract)
                    nc.vector.tensor_tensor(m[:, 0:2, :], m[:, 0:2, :],
                                            m[:, 2:4, :], op=Op.add)
                    nc.vector.tensor_tensor(m[:, 0, :], m[:, 0, :],
                                            m[:, 1, :], op=Op.add)
                    nc.vector.tensor_tensor(pq[:, 0:3, :], pq[:, 0:3, :],
                                            pq[:, 3:6, :], op=Op.add)
                    nc.vector.tensor_tensor(pq[:, 0, :], pq[:, 0, :],
                                            pq[:, 1, :], op=Op.add)
                    nc.vector.tensor_tensor(m[:, 1, :], pq[:, 0, :],
                                            pq[:, 2, :], op=Op.add)

                    # one fused square+accum per chunk (wr,wi x both dirs),
                    # squared in place over the fold results
                    nc.scalar.activation(m[:, 0:2, :], m[:, 0:2, :], Act.Square,
                                         accum_out=acc[:, ch:ch + 1])

                st2.close()

            # ======== final reduce + affine ========
            stot = small_pool.tile([PB, 1], f32)
            nc.vector.tensor_reduce(stot[:], acc[:], axis=X, op=Op.add)
            res = small_pool.tile([PB, 1], f32)
            nc.vector.tensor_scalar(res[:], stot[:], -NBETA, NBETA * 2.0 * S,
                                    op0=Op.mult, op1=Op.add)
            nc.sync.dma_start(out_d[:], res[:])

    nc.finalize()
    return nc


def kernel(phi, shift):
    from concourse.bass_utils import run_bass_kernel_spmd

    phi = np.ascontiguousarray(np.asarray(phi, dtype=np.float32))
    shift = np.asarray(shift, dtype=np.int32)
    key = (shift.tobytes(), 1)
    if key not in _cache:
        _cache[key] = _build(shift)
    nc = _cache[key]

    in_maps = [{"phi": phi[i * PB:(i + 1) * PB]} for i in range(NCORES)]
    res = run_bass_kernel_spmd(nc, in_maps, core_ids=list(range(NCORES)))
    out = np.concatenate([r["out"] for r in res.results], axis=0)
    return out.astype(np.float32)



# revision 6
# speedup vs baseline: 1.8321x; 1.8321x over previous
"""CP(n) lattice action kernel for Trainium2 (8 NeuronCores, Bass/Tile).

Math (matches reference):
  phi: (B=1024, S=4096, n=6) angles; shift: (2, S) neighbor site indices.
  Wrap: first 5 angles mod pi, last mod 2pi.
  x = hyperspherical embedding (7 comps); z = (x0..x3) + i(x4,x5,x6,0).
  w_d(s) = sum_k z_k(s) z_k(shift[d,s])
  action[b] = -4 * sum_{d,s} (|w_d(s)|^2 - 1)

Implementation (v2):
  - Pure data parallel: batch axis sharded 8 x 128 (batch on partitions).
  - Range reduction: k = RNE(phi/2pi) (DVE fp32->int16 convert), r = phi -
    2pi*k in [-pi, pi] (DVE stt, in place over phi).
  - Trig via ScalarE Sin with angle-major strided reads (the ONLY strided
    ops; DVE strided access measured 7x slower than contiguous, ACT only
    2.2x): t = sin(r), u = Sin(-r + pi/2) = cos(r) via the free affine.
  - mod-pi wrap for angles 0..4 via bitwise ops on bf16: s_j = |t|,
    c_j = u XOR signbit(t); angle 5 (mod 2pi) uses t, u raw.
  - x built angle-major with contiguous bf16 TT ops (2x DVE mode) into a
    padded lattice layout: rows of 66 (64 cols + wrap halo col + pad), plus
    a halo row replicating row 0. Both neighbor shifts become pure offset
    reads: +66 (down), +1 (right).
  - Neighbor products / folds: big multi-plane TT ops; squares + site
    reduction fused into ACT Square accum_out.
"""
import contextlib
import sys

import numpy as np

sys.path.insert(0, "/opt/trn_rl_repo")

B, S, NA = 1024, 4096, 6
NCORES = 8
PB = B // NCORES          # 128 batches per core
L = 64                    # lattice row length
C1 = 1024                 # stage-1 site chunk
N1 = S // C1
C2 = 2048                 # stage-2 site chunk
N2 = S // C2
RW = 66                   # padded row width (64 + halo col + pad)
NROWH = L + 1             # 64 rows + halo row
CSTRIDE = 4352            # per-component stride in xpad (>= 65*66 = 4290)
PI = float(np.pi)
NBETA = 4.0               # N * BETA

_cache = {}


def _detect_roll(shift):
    idx = np.arange(S).reshape(L, L)
    s0 = np.roll(idx, -1, axis=0).ravel()
    s1 = np.roll(idx, -1, axis=1).ravel()
    return np.array_equal(shift[0], s0) and np.array_equal(shift[1], s1)


def _build(shift, reps=1, mode="full"):
    if not _detect_roll(shift):
        return _build_fallback(shift, reps=reps)

    import concourse.bass as bass
    import concourse.tile as tile
    from concourse import bacc, mybir

    f32 = mybir.dt.float32
    bf16 = mybir.dt.bfloat16
    i16 = mybir.dt.int16
    Act = mybir.ActivationFunctionType
    Op = mybir.AluOpType
    X = mybir.AxisListType.X

    nc = bacc.Bacc(None, target_bir_lowering=False)
    phi_d = nc.dram_tensor("phi", [PB, S, NA], f32, kind="ExternalInput")
    out_d = nc.dram_tensor("out", [PB, 1], f32, kind="ExternalOutput")
    pd_flat = phi_d[:].rearrange("p s a -> p (s a)")

    with tile.TileContext(nc) as tc:
        with contextlib.ExitStack() as ctx:
            xpool = ctx.enter_context(tc.tile_pool(name="xpad", bufs=1))
            small = ctx.enter_context(tc.tile_pool(name="small", bufs=1))

            xpad = xpool.tile([PB, 7, CSTRIDE], bf16)
            acc = small.tile([PB, 2 * N2], f32)
            half_pi = small.tile([PB, 1], f32)
            nc.vector.memset(half_pi[:], PI / 2)
            xap = xpad[:]

            def xcomp(k0, n, off, nrow):
                # [n comps][nrow rows][64 cols] view of xpad
                return bass.AP(
                    tensor=xap.tensor, offset=xap.offset + k0 * CSTRIDE + off,
                    ap=[xap.ap[0], [CSTRIDE, n], [RW, nrow], [1, L]])

            for rep in range(reps):
                # ======== stage 1: wrap + trig + embedding ========
                st1 = contextlib.ExitStack()
                p_phi = st1.enter_context(tc.tile_pool(name="p_phi", bufs=2))
                p_k = st1.enter_context(tc.tile_pool(name="p_k", bufs=1))
                p_tu = st1.enter_context(tc.tile_pool(name="p_tu", bufs=2))
                p_cum = st1.enter_context(tc.tile_pool(name="p_cum", bufs=1))

                for ch in range(N1):
                    cs = ch * C1
                    M = C1 * NA
                    nrow = C1 // L
                    rb = (cs // L) * RW

                    phic = p_phi.tile([PB, M], f32, tag="phic")
                    nc.sync.dma_start(phic[:], pd_flat[:, cs * NA:(cs + C1) * NA])

                    if mode == "dma":
                        nc.vector.tensor_reduce(acc[:, 0:1], phic[:, 0:8],
                                                axis=X, op=Op.add)
                        continue

                    # k = RNE(phi/2pi); r = phi - 2pi*k  (in place)
                    k = p_k.tile([PB, M], i16, tag="k")
                    nc.vector.tensor_scalar(k[:], phic[:], 1.0 / (2 * PI),
                                            None, op0=Op.mult)
                    nc.vector.scalar_tensor_tensor(
                        phic[:], k[:], -2 * PI, phic[:],
                        op0=Op.mult, op1=Op.add)

                    # angle-major trig: t = sin(r), u = cos(r)
                    t = p_tu.tile([PB, 6, C1], bf16, tag="t")
                    u = p_tu.tile([PB, 6, C1], bf16, tag="u")
                    pap = phic[:]
                    src = bass.AP(tensor=pap.tensor, offset=pap.offset,
                                  ap=[pap.ap[0], [1, NA], [NA, C1]])
                    nc.scalar.activation(t[:], src, Act.Sin)
                    nc.scalar.activation(u[:], src, Act.Sin,
                                         scale=-1.0, bias=half_pi[:])

                    # angles 0..4: s = |t|, c = u ^ signbit(t)
                    t5 = t[:, 0:5, :].bitcast(i16)
                    u5 = u[:, 0:5, :].bitcast(i16)
                    sgn = p_k.tile([PB, 5 * C1], i16, tag="sgn")
                    nc.vector.tensor_scalar(sgn[:], t5, -0x8000, None,
                                            op0=Op.bitwise_and)
                    nc.vector.tensor_scalar(t5, t5, 0x7FFF, None,
                                            op0=Op.bitwise_and)
                    nc.vector.tensor_tensor(u5, u5, sgn[:],
                                            op=Op.bitwise_xor)

                    # x build (angle-major, contiguous) into padded layout
                    def tv(tile_, j):
                        a = tile_[:, j, :]
                        return bass.AP(tensor=a.tensor, offset=a.offset,
                                       ap=[a.ap[0], [L, nrow], [1, L]])

                    def xp(kc):
                        return xcomp(kc, 1, rb, nrow)

                    cA = p_cum.tile([PB, C1], bf16, tag="cA")
                    cB = p_cum.tile([PB, C1], bf16, tag="cB")
                    cC = p_cum.tile([PB, C1], bf16, tag="cC")
                    cD = p_cum.tile([PB, C1], bf16, tag="cD")
                    cv = lambda c_: bass.AP(tensor=c_.tensor,
                                            offset=c_[:].offset,
                                            ap=[c_[:].ap[0], [L, nrow], [1, L]])
                    nc.vector.tensor_copy(xp(0), tv(u, 0))
                    nc.vector.tensor_tensor(xp(1), tv(u, 1), tv(t, 0),
                                            op=Op.mult)
                    nc.vector.tensor_tensor(cA[:], t[:, 0, :], t[:, 1, :],
                                            op=Op.mult)
                    nc.vector.tensor_tensor(xp(2), tv(u, 2), cv(cA),
                                            op=Op.mult)
                    nc.vector.tensor_tensor(cB[:], cA[:], t[:, 2, :],
                                            op=Op.mult)
                    nc.vector.tensor_tensor(xp(3), tv(u, 3), cv(cB),
                                            op=Op.mult)
                    nc.vector.tensor_tensor(cC[:], cB[:], t[:, 3, :],
                                            op=Op.mult)
                    nc.vector.tensor_tensor(xp(4), tv(u, 4), cv(cC),
                                            op=Op.mult)
                    nc.vector.tensor_tensor(cD[:], cC[:], t[:, 4, :],
                                            op=Op.mult)
                    nc.vector.tensor_tensor(xp(5), tv(u, 5), cv(cD),
                                            op=Op.mult)
                    nc.vector.tensor_tensor(xp(6), cv(cD), tv(t, 5),
                                            op=Op.mult)

                st1.close()
                if mode == "dma":
                    continue

                # halo fills: wrap col (col 64 <- col 0), wrap row
                nc.vector.tensor_copy(
                    bass.AP(tensor=xap.tensor, offset=xap.offset + L,
                            ap=[xap.ap[0], [CSTRIDE, 7], [RW, L]]),
                    bass.AP(tensor=xap.tensor, offset=xap.offset,
                            ap=[xap.ap[0], [CSTRIDE, 7], [RW, L]]))
                nc.vector.tensor_copy(
                    bass.AP(tensor=xap.tensor, offset=xap.offset + L * RW,
                            ap=[xap.ap[0], [CSTRIDE, 7], [1, L + 1]]),
                    bass.AP(tensor=xap.tensor, offset=xap.offset,
                            ap=[xap.ap[0], [CSTRIDE, 7], [1, L + 1]]))

                # ======== stage 2: neighbor products ========
                st2 = contextlib.ExitStack()
                p_P = st2.enter_context(tc.tile_pool(name="p_P", bufs=1))

                for ch in range(N2):
                    nrow = C2 // L
                    rb = ch * nrow * RW
                    P = p_P.tile([PB, 13, 2 * C2], bf16, tag="P")
                    Pap = P[:]

                    def pv(k0, n, half):
                        return bass.AP(
                            tensor=Pap.tensor,
                            offset=Pap.offset + k0 * 2 * C2 + half * C2,
                            ap=[Pap.ap[0], [2 * C2, n], [L, nrow], [1, L]])

                    for d, off in ((0, RW), (1, 1)):
                        nc.vector.tensor_tensor(
                            pv(0, 7, d), xcomp(0, 7, rb, nrow),
                            xcomp(0, 7, rb + off, nrow), op=Op.mult)
                        nc.vector.tensor_tensor(
                            pv(7, 3, d), xcomp(0, 3, rb, nrow),
                            xcomp(4, 3, rb + off, nrow), op=Op.mult)
                        nc.vector.tensor_tensor(
                            pv(10, 3, d), xcomp(4, 3, rb, nrow),
                            xcomp(0, 3, rb + off, nrow), op=Op.mult)

                    # folds: dre = P0+P1+P2+P3-P4-P5-P6 ; dim = sum P7..P12
                    nc.vector.tensor_tensor(P[:, 0:3, :], P[:, 0:3, :],
                                            P[:, 4:7, :], op=Op.subtract)
                    nc.vector.tensor_tensor(P[:, 0:2, :], P[:, 0:2, :],
                                            P[:, 2:4, :], op=Op.add)
                    nc.vector.tensor_tensor(P[:, 0, :], P[:, 0, :],
                                            P[:, 1, :], op=Op.add)
                    nc.vector.tensor_tensor(P[:, 7:10, :], P[:, 7:10, :],
                                            P[:, 10:13, :], op=Op.add)
                    nc.vector.tensor_tensor(P[:, 7, :], P[:, 7, :],
                                            P[:, 8, :], op=Op.add)
                    nc.vector.tensor_tensor(P[:, 7, :], P[:, 7, :],
                                            P[:, 9, :], op=Op.add)

                    nc.scalar.activation(P[:, 0, :], P[:, 0, :], Act.Square,
                                         accum_out=acc[:, 2 * ch:2 * ch + 1])
                    nc.scalar.activation(P[:, 7, :], P[:, 7, :], Act.Square,
                                         accum_out=acc[:, 2 * ch + 1:2 * ch + 2])

                st2.close()

            # ======== final reduce + affine ========
            stot = small.tile([PB, 1], f32)
            nc.vector.tensor_reduce(stot[:], acc[:], axis=X, op=Op.add)
            res = small.tile([PB, 1], f32)
            nc.vector.tensor_scalar(res[:], stot[:], -NBETA, NBETA * 2.0 * S,
                                    op0=Op.mult, op1=Op.add)
            nc.sync.dma_start(out_d[:], res[:])

    nc.finalize()
    return nc


def _build_fallback(shift, reps=1):
    """General-shift path (gpsimd ap_gather), from the v1 kernel."""
    import concourse.bass as bass
    import concourse.tile as tile
    from concourse import bacc, mybir

    f32 = mybir.dt.float32
    bf16 = mybir.dt.bfloat16
    i16 = mybir.dt.int16
    Act = mybir.ActivationFunctionType
    Op = mybir.AluOpType
    X = mybir.AxisListType.X

    FC1 = 2048
    FN1 = S // FC1
    FC2 = 2048
    FN2 = S // FC2

    nc = bacc.Bacc(None, target_bir_lowering=False)
    phi_d = nc.dram_tensor("phi", [PB, S, NA], f32, kind="ExternalInput")
    out_d = nc.dram_tensor("out", [PB, 1], f32, kind="ExternalOutput")
    pd_flat = phi_d[:].rearrange("p s a -> p (s a)")

    with tile.TileContext(nc) as tc:
        with contextlib.ExitStack() as ctx:
            xfull_pool = ctx.enter_context(tc.tile_pool(name="xfull", bufs=1))
            small_pool = ctx.enter_context(tc.tile_pool(name="small", bufs=1))

            acc = small_pool.tile([PB, FN2], f32)
            xg = xfull_pool.tile([PB, S, 8], bf16)
            idx_sb = []
            for d in range(2):
                wrapped = np.zeros((PB, S // 16), np.int16)
                base = shift[d].reshape(S // 16, 16).T.astype(np.int16)
                for g in range(PB // 16):
                    wrapped[16 * g:16 * (g + 1)] = base
                hdl = nc.inline_tensor(wrapped, name=f"shift_idx_{d}")
                t_ = small_pool.tile([PB, S // 16], mybir.dt.int16,
                                     tag=f"idx{d}")
                nc.sync.dma_start(t_[:], hdl[:])
                idx_sb.append(t_)

            for rep in range(reps):
                st1 = contextlib.ExitStack()
                p_phi = st1.enter_context(tc.tile_pool(name="p_phi", bufs=1))
                p_ks = st1.enter_context(tc.tile_pool(name="p_ks", bufs=1))
                p_t = st1.enter_context(tc.tile_pool(name="p_t", bufs=1))
                p_u = st1.enter_context(tc.tile_pool(name="p_u", bufs=1))
                p_cum = st1.enter_context(tc.tile_pool(name="p_cum", bufs=1))

                for ch in range(FN1):
                    cs = ch * FC1
                    M = FC1 * NA

                    phic = p_phi.tile([PB, M], f32, tag="phic")
                    nc.sync.dma_start(phic[:],
                                      pd_flat[:, cs * NA:(cs + FC1) * NA])

                    k = p_ks.tile([PB, M], i16, tag="ks")
                    nc.vector.tensor_scalar(k[:], phic[:], 1.0 / (2 * PI),
                                            None, op0=Op.mult)
                    nc.vector.scalar_tensor_tensor(
                        phic[:], k[:], -2 * PI, phic[:], op0=Op.mult,
                        op1=Op.add)

                    sig = p_ks.tile([PB, M], bf16, tag="ks")
                    nc.scalar.activation(sig[:], phic[:], Act.Sign)
                    tt = p_t.tile([PB, M], bf16, tag="t")
                    nc.scalar.activation(tt[:], phic[:], Act.Sin)
                    nc.vector.scalar_tensor_tensor(
                        phic[:], sig[:], PI / 2, phic[:],
                        op0=Op.mult, op1=Op.subtract)
                    uu = p_u.tile([PB, M], bf16, tag="u")
                    nc.scalar.activation(uu[:], phic[:], Act.Sin)

                    def ang(tile_, j, n=1):
                        ap = tile_[:]
                        if n == 1:
                            return bass.AP(tensor=ap.tensor,
                                           offset=ap.offset + j,
                                           ap=[ap.ap[0], [NA, FC1]])
                        return bass.AP(tensor=ap.tensor, offset=ap.offset + j,
                                       ap=[ap.ap[0], [NA, FC1], [1, n]])

                    nc.vector.tensor_tensor(ang(tt, 0, 5), ang(tt, 0, 5),
                                            ang(sig, 0, 5), op=Op.mult)
                    nc.vector.tensor_tensor(ang(uu, 5), ang(uu, 5),
                                            ang(sig, 5), op=Op.mult)

                    cumA = p_cum.tile([PB, FC1], bf16, tag="cumA")
                    cumB = p_cum.tile([PB, FC1], bf16, tag="cumB")
                    gap = xg[:]
                    xk = [bass.AP(tensor=gap.tensor,
                                  offset=gap.offset + cs * 8 + kk,
                                  ap=[gap.ap[0], [8, FC1]])
                          for kk in range(7)]
                    nc.vector.tensor_copy(xk[0], ang(uu, 0))
                    nc.vector.tensor_tensor(xk[1], ang(uu, 1), ang(tt, 0),
                                            op=Op.mult)
                    nc.vector.tensor_tensor(cumA[:], ang(tt, 0), ang(tt, 1),
                                            op=Op.mult)
                    nc.vector.tensor_tensor(xk[2], ang(uu, 2), cumA[:],
                                            op=Op.mult)
                    nc.vector.tensor_tensor(cumB[:], cumA[:], ang(tt, 2),
                                            op=Op.mult)
                    nc.vector.tensor_tensor(xk[3], ang(uu, 3), cumB[:],
                                            op=Op.mult)
                    nc.vector.tensor_tensor(cumA[:], cumB[:], ang(tt, 3),
                                            op=Op.mult)
                    nc.vector.tensor_tensor(xk[4], ang(uu, 4), cumA[:],
                                            op=Op.mult)
                    nc.vector.tensor_tensor(cumB[:], cumA[:], ang(tt, 4),
                                            op=Op.mult)
                    nc.vector.tensor_tensor(xk[5], ang(uu, 5), cumB[:],
                                            op=Op.mult)
                    nc.vector.tensor_tensor(xk[6], cumB[:], ang(tt, 5),
                                            op=Op.mult)

                st1.close()

                st2 = contextlib.ExitStack()
                p_xp = st2.enter_context(tc.tile_pool(name="p_xp", bufs=1))
                p_m = st2.enter_context(tc.tile_pool(name="p_m", bufs=1))
                p_pq = st2.enter_context(tc.tile_pool(name="p_pq", bufs=1))

                for ch in range(FN2):
                    cs = ch * FC2
                    gap = xg[:]
                    xg_k = lambda k0, n: bass.AP(
                        tensor=gap.tensor, offset=gap.offset + cs * 8 + k0,
                        ap=[gap.ap[0], [1, n], [8, FC2]])

                    m = p_m.tile([PB, 7, 2 * FC2], bf16, tag="m")
                    pq = p_pq.tile([PB, 6, 2 * FC2], bf16, tag="pq")

                    for d in (0, 1):
                        xpg = p_xp.tile([PB, FC2, 8], bf16, tag="xp")
                        nc.gpsimd.ap_gather(
                            xpg[:], xg[:],
                            idx_sb[d][:, cs // 16:(cs + FC2) // 16],
                            channels=PB, num_elems=S, d=8, num_idxs=FC2)
                        gp = xpg[:]
                        xp_k = lambda k0, n: bass.AP(
                            tensor=gp.tensor, offset=gp.offset + k0,
                            ap=[gp.ap[0], [1, n], [8, FC2]])

                        ms = m[:, :, d * FC2:(d + 1) * FC2]
                        pqs = pq[:, :, d * FC2:(d + 1) * FC2]
                        nc.vector.tensor_tensor(ms, xg_k(0, 7), xp_k(0, 7),
                                                op=Op.mult)
                        nc.vector.tensor_tensor(pqs[:, 0:3, :], xg_k(0, 3),
                                                xp_k(4, 3), op=Op.mult)
                        nc.vector.tensor_tensor(pqs[:, 3:6, :], xg_k(4, 3),
                                                xp_k(0, 3), op=Op.mult)

                    nc.vector.tensor_tensor(m[:, 0:3, :], m[:, 0:3, :],
                                            m[:, 4:7, :], op=Op.subtract)
                    nc.vector.tensor_tensor(m[:, 0:2, :], m[:, 0:2, :],
                                            m[:, 2:4, :], op=Op.add)
                    nc.vector.tensor_tensor(m[:, 0, :], m[:, 0, :],
                                            m[:, 1, :], op=Op.add)
                    nc.vector.tensor_tensor(pq[:, 0:3, :], pq[:, 0:3, :],
                                            pq[:, 3:6, :], op=Op.add)
                    nc.vector.tensor_tensor(pq[:, 0, :], pq[:, 0, :],
                                            pq[:, 1, :], op=Op.add)
                    nc.vector.tensor_tensor(m[:, 1, :], pq[:, 0, :],
                                            pq[:, 2, :], op=Op.add)

                    nc.scalar.activation(m[:, 0:2, :], m[:, 0:2, :],
                                         Act.Square,
                                         accum_out=acc[:, ch:ch + 1])

                st2.close()

            stot = small_pool.tile([PB, 1], f32)
            nc.vector.tensor_reduce(stot[:], acc[:], axis=X, op=Op.add)
            res = small_pool.tile([PB, 1], f32)
            nc.vector.tensor_scalar(res[:], stot[:], -NBETA, NBETA * 2.0 * S,
                                    op0=Op.mult, op1=Op.add)
            nc.sync.dma_start(out_d[:], res[:])

    nc.finalize()
    return nc


def kernel(phi, shift):
    from concourse.bass_utils import run_bass_kernel_spmd

    phi = np.ascontiguousarray(np.asarray(phi, dtype=np.float32))
    shift = np.asarray(shift, dtype=np.int32)
    key = (shift.tobytes(), 1)
    if key not in _cache:
        _cache[key] = _build(shift)
    nc = _cache[key]

    in_maps = [{"phi": phi[i * PB:(i + 1) * PB]} for i in range(NCORES)]
    res = run_bass_kernel_spmd(nc, in_maps, core_ids=list(range(NCORES)))
    out = np.concatenate([r["out"] for r in res.results], axis=0)
    return out.astype(np.float32)
